# revision 15
# baseline (speedup 1.0000x reference)
"""Trainium2 Bass kernel for nn_CommunicationLayer (gnn_message_passing).

Computes, for A=3 agents over batch B with feature dim D=128:
    total       = sum_a x_a                      # [1, B, D]
    mean_others = (total - x_i) / (A-1)          # [A, B, D]
    out_i       = x_i + mean_others_i @ W + b    # [A, B, D]

The problem is HBM-bandwidth bound. The correctness gate is rel_err <
2e-2 and fp16 carries ~3e-4 relative error through this computation, so
all device I/O runs in fp16 (host casts x and the weights, and casts the
fp16 result back to fp32), halving the DMA roofline to ~281us/core.

Algebra: with W' = W/(A-1),
    out_i = x_i + (S - x_i) @ W' = S @ W' + x_i @ (I - W'),  S = sum_a x_a
so per 128-row group the kernel runs exactly 4 matmuls and nothing else:
one with stationary S^T and moving [W'|W'|W'] (broadcasts T = S@W' into
all three agent blocks of the group's PSUM tile), plus one per agent
with stationary x_i^T and moving (I - W') accumulating into block i.
Each group's PSUM tile is [128, A*D] fp32 = 1.5KB, laid out (i d)
contiguous: a matmul's output columns must stay inside one 2KB PSUM
bank (multi-bank scatter writes scramble), so the N=384 broadcast
matmul writes columns 0..383 of a single bank.

x^T arrives for free: fp16 enables the HWDGE xbar DMA-transpose path
(~225GB/s effective, under the 281us wall), so loads land feature-major
directly and the kernel has zero PE transposes and no tpsum round-trip.
S costs two DVE adds per chunk. Evacuation PSUM->SBUF (cast fp32->fp16)
is split 5:3 between ACT and DVE, and each half-chunk stores via the
otherwise-idle GPSIMD sequencer (SWDGE) so the SP load stream never
blocks behind store data dependencies.

Distribution: data-parallel over the batch axis across 8 NeuronCores
(no cross-device communication), weights replicated.
"""

import numpy as np

import concourse.bacc as bacc
import concourse.bass as bass  # noqa: F401
import concourse.mybir as mybir
from concourse.tile import TileContext
from concourse.bass_utils import run_bass_kernel_spmd

A = 3
B = 524288
D = 128
NCORES = 8
BC = B // NCORES          # 65536 batch rows per core
CHUNK = 4096              # batch rows per chunk
W_PER = CHUNK // 128      # 32 groups of 128 rows per chunk
NCHUNK = BC // CHUNK      # 16

F32 = mybir.dt.float32
F16 = mybir.dt.float16


def build_bass():
    # Bacc (not plain Bass): its compile pipeline moves matmul waits onto
    # ldweights and splits >1-wait sync conditions into event semaphores.
    nc = bacc.Bacc(None, target_bir_lowering=False)

    x_ext = nc.declare_dram_parameter("x", [A, BC, D], F16, isOutput=False)
    w3_ext = nc.declare_dram_parameter("w3", [D, A * D], F16, isOutput=False)
    wi_ext = nc.declare_dram_parameter("wi", [D, D], F16, isOutput=False)
    y_ext = nc.declare_dram_parameter("y", [A, BC, D], F16, isOutput=True)

    with TileContext(nc) as tc:
        with (
            tc.tile_pool(name="const", bufs=1) as cpool,
            tc.tile_pool(name="xt_pool", bufs=12) as in_pool,
            tc.tile_pool(name="s_pool", bufs=3) as s_pool,
            tc.tile_pool(name="xout_pool", bufs=4) as out_pool,
            tc.tile_pool(name="mpsum_pool", bufs=4, space="PSUM") as mpsum_pool,
        ):
            w3 = cpool.tile([D, A * D], F16)
            nc.sync.dma_start(out=w3, in_=w3_ext[:, :])
            wi = cpool.tile([D, D], F16)
            nc.sync.dma_start(out=wi, in_=wi_ext[:, :])

            for c in range(NCHUNK):
                b0 = c * CHUNK
                # Feature-major x^T for this chunk: one [d, b] tile per
                # agent via the xbar DMA-transpose path (source is a
                # contiguous [CHUNK, D] block per agent; all transposes on
                # the SP ring — concurrent xbar streams from two HWDGE
                # rings corrupt). Per-agent tiles release independently.
                # The host pre-permuted rows w-major (row m*W_PER+g sits at
                # source position g*128+m), so tile column g*128+m is batch
                # row m*W_PER+g: group g is the contiguous column block
                # [g*128, (g+1)*128) AND lines up with the (p w)-factored
                # 4KB-contiguous store below; ldweights slices stay
                # contiguous (FWL-eligible).
                xTs = []
                for a in range(A):
                    xTa = in_pool.tile([128, CHUNK], F16, tag="xT")
                    nc.sync.dma_start_transpose(
                        out=xTa, in_=x_ext[a, b0:b0 + CHUNK, :],
                    )
                    xTs.append(xTa)

                st = s_pool.tile([128, CHUNK], F16, tag="st")
                nc.vector.tensor_add(out=st, in0=xTs[0], in1=xTs[1])
                nc.vector.tensor_add(out=st, in0=st, in1=xTs[2])

                for h in range(2):
                    # Per-half-chunk output tile, stored on GPSIMD/SWDGE.
                    xoh = out_pool.tile([128, A * 16 * D], F16, tag="xout")
                    xoh4 = xoh.rearrange("p (a w d) -> p a w d", a=A, d=D)
                    for lp in range(8):
                        pair = 8 * h + lp
                        # Two groups share one padded 2-bank PSUM tile:
                        # even group at cols 0:384 (bank 0), odd at 512:896
                        # (bank 1) — each matmul's output stays inside one
                        # 2KB PSUM bank.
                        ps = mpsum_pool.tile([128, 1024], F32, tag="ps")
                        for sub in range(2):
                            g = 2 * pair + sub
                            off = 512 * sub
                            # T = S @ W' broadcast into all three blocks.
                            nc.tensor.matmul(
                                ps[:, off:off + A * D],
                                lhsT=st[:, g * 128:(g + 1) * 128],
                                rhs=w3,
                                start=True,
                                stop=False,
                                skip_group_check=True,
                            )
                            # Block i += x_i @ (I - W')  (residual fused).
                            for j in range(A):
                                nc.tensor.matmul(
                                    ps[:, off + j * D:off + (j + 1) * D],
                                    lhsT=xTs[j][:, g * 128:(g + 1) * 128],
                                    rhs=wi,
                                    start=False,
                                    stop=True,
                                    skip_group_check=True,
                                )
                        # Evacuate both groups in one op. DVE takes only
                        # the 4 earliest pairs of each chunk so its stream
                        # frees up for the next chunk's S-adds mid-chunk;
                        # ACT (which has no other work) takes the rest.
                        dst = xoh4[:, :, 2 * lp:2 * lp + 2, :]
                        src = ps.rearrange("p (w q) -> p w q", w=2)[
                            :, :, 0:A * D
                        ].rearrange("p w (i d) -> p i w d", d=D)
                        if pair < 4:
                            nc.vector.tensor_copy(out=dst, in_=src)
                        else:
                            nc.scalar.copy(out=dst, in_=src)

                    dst = y_ext[:, b0:b0 + CHUNK, :].rearrange(
                        "a (p w) d -> p a w d", p=128
                    )[:, :, 16 * h:16 * h + 16, :]
                    nc.gpsimd.dma_start(out=dst, in_=xoh4)

    # Bacc defers register allocation to its compile() pass (run by
    # finalize); the PJRT exec path serializes nc as-is, so finalize here.
    nc.finalize()
    return nc


def run(inputs, trace=False):
    """Build, compile, and run on 8 cores. Returns (full_output, results_obj)."""
    agent_states = np.asarray(inputs["agent_states"], dtype=np.float32)
    W = np.asarray(inputs["W"], dtype=np.float32)
    b = np.asarray(inputs["b"], dtype=np.float32)

    wp = W * (1.0 / (A - 1))
    w3_host = np.ascontiguousarray(
        np.concatenate([wp, wp, wp], axis=1).astype(np.float16)
    )
    wi_host = np.ascontiguousarray((np.eye(D, dtype=np.float32) - wp)
                                   .astype(np.float16))
    x16 = agent_states.astype(np.float16)

    nc = build_bass()

    in_maps = []
    for i in range(NCORES):
        shard = x16[:, i * BC:(i + 1) * BC, :]
        # Pre-permute each chunk's rows w-major (row m*W_PER+w -> position
        # w*128+m) so the on-device transpose-load lands group-contiguous
        # while the store keeps its (p w)-factored 4KB runs.
        shard = np.ascontiguousarray(
            shard.reshape(A, NCHUNK, 128, W_PER, D)
                 .transpose(0, 1, 3, 2, 4)
                 .reshape(A, BC, D)
        )
        in_maps.append({"x": shard, "w3": w3_host, "wi": wi_host})

    res = run_bass_kernel_spmd(nc, in_maps, list(range(NCORES)), trace=trace)

    out = np.concatenate([r["y"] for r in res.results], axis=1).astype(np.float32)
    if np.any(b):
        out = out + b.reshape(1, 1, D)
    return out, res


def kernel(**inputs):
    out, _ = run(inputs, trace=False)
    return out
